# revision 1
# baseline (speedup 1.0000x reference)
"""Trainium2 Bass kernel for nn_CGCoupler (segment_reduce).

The CG coupling tables have a fixed block structure: every index triple
(repids_in1, repids_in2, repids_out) decomposes into 147 block-ops

    out[:, bo*64:(bo+1)*64] += c * x1[:, b1*64:(b1+1)*64] * x2[:, b2*64:(b2+1)*64]

with 64-aligned contiguous blocks (verified against build_tables for
metadata=[64,64,64,64], overlap_out=True, trunc_in=True). The kernel hardcodes
the (b1, b2, bo) structure and takes the coefficient values from the runtime
cg_tilde input. Data-parallel over rows: 4096 rows -> 8 cores x 512 rows.
"""
import numpy as np

# (b1, b2, bo) block triples sorted by (bo, b1, b2); ORDER maps each sorted slot
# to its row in the original build_tables op order (cg_tilde.reshape(147, 64)).
OPS = [
    (0,0,0),(1,1,0),(2,2,0),(3,3,0),
    (0,1,1),(1,0,1),(1,6,1),(1,8,1),(2,3,1),(2,5,1),(3,2,1),(3,4,1),(4,3,1),(5,2,1),(6,1,1),(8,1,1),
    (0,2,2),(1,3,2),(1,5,2),(2,0,2),(2,6,2),(3,1,2),(3,7,2),(5,1,2),(6,2,2),(7,3,2),
    (0,3,3),(1,2,3),(1,4,3),(2,1,3),(2,7,3),(3,0,3),(3,6,3),(3,8,3),(4,1,3),(6,3,3),(7,2,3),(8,3,3),
    (0,4,4),(1,3,4),(1,5,4),(2,8,4),(3,1,4),(3,7,4),(4,0,4),(5,1,4),(7,3,4),(8,2,4),
    (0,5,5),(1,2,5),(1,4,5),(2,1,5),(2,7,5),(3,6,5),(3,8,5),(4,1,5),(5,0,5),(6,3,5),(7,2,5),(8,3,5),
    (0,6,6),(1,1,6),(1,7,6),(2,2,6),(3,3,6),(3,5,6),(5,3,6),(6,0,6),(7,1,6),
    (0,7,7),(1,6,7),(1,8,7),(2,3,7),(2,5,7),(3,2,7),(3,4,7),(4,3,7),(5,2,7),(6,1,7),(7,0,7),(8,1,7),
    (0,8,8),(1,1,8),(1,7,8),(2,4,8),(3,3,8),(3,5,8),(4,2,8),(5,3,8),(7,1,8),(8,0,8),
    (0,9,9),(1,8,9),(3,4,9),(4,3,9),(8,1,9),(9,0,9),
    (0,10,10),(1,7,10),(2,4,10),(3,5,10),(4,2,10),(5,3,10),(7,1,10),(10,0,10),
    (0,11,11),(1,6,11),(1,8,11),(2,5,11),(3,4,11),(4,3,11),(5,2,11),(6,1,11),(8,1,11),(11,0,11),
    (0,12,12),(1,5,12),(2,6,12),(3,7,12),(5,1,12),(6,2,12),(7,3,12),(12,0,12),
    (0,13,13),(1,4,13),(2,7,13),(3,6,13),(3,8,13),(4,1,13),(6,3,13),(7,2,13),(8,3,13),(13,0,13),
    (0,14,14),(1,5,14),(2,8,14),(3,7,14),(5,1,14),(7,3,14),(8,2,14),(14,0,14),
    (0,15,15),(1,4,15),(3,8,15),(4,1,15),(8,3,15),(15,0,15),
]
N_OPS = len(OPS)
N_CORES = 8
ROWS_PER_CORE = 512
D = 1024


def _runs():
    """Maximal constant-delta runs within each bo segment (bo constant, slot+1)."""
    runs = []
    i = 0
    while i < N_OPS:
        b1, b2, bo = OPS[i]
        j = i + 1
        if j < N_OPS and OPS[j][2] == bo:
            d1 = OPS[j][0] - OPS[j - 1][0]
            d2 = OPS[j][1] - OPS[j - 1][1]
            while (j + 1 < N_OPS and OPS[j + 1][2] == bo
                   and OPS[j + 1][0] - OPS[j][0] == d1
                   and OPS[j + 1][1] - OPS[j][1] == d2):
                j += 1
            runs.append((i, j - i + 1 if j > i else 1, d1, d2))
            i = j + 1
        else:
            runs.append((i, 1, 0, 0))
            i = j
    return runs


RUNS = _runs()
SEG = []
_i = 0
for _bo in range(16):
    _n = sum(1 for o in OPS if o[2] == _bo)
    SEG.append((_i, _n))
    _i += _n

_CACHE = {}


def _build():
    from concourse import bacc, mybir
    import concourse.tile as tile

    f32 = mybir.dt.float32
    nc = bacc.Bacc("TRN2", target_bir_lowering=False)
    x1_d = nc.dram_tensor("x1", [ROWS_PER_CORE, D], f32, kind="ExternalInput")
    x2_d = nc.dram_tensor("x2", [ROWS_PER_CORE, D], f32, kind="ExternalInput")
    cg_d = nc.dram_tensor("cgrow", [1, N_OPS * 64], f32, kind="ExternalInput")
    out_d = nc.dram_tensor("out", [ROWS_PER_CORE, D], f32, kind="ExternalOutput")

    with tile.TileContext(nc) as tc:
        with (
            tc.tile_pool(name="const", bufs=1) as constp,
            tc.tile_pool(name="io", bufs=2) as iop,
            tc.tile_pool(name="spp", bufs=1) as spp,
        ):
            cgrow = constp.tile([1, N_OPS * 64], f32)
            nc.sync.dma_start(cgrow[:], cg_d[:])
            crep = constp.tile([128, N_OPS * 64], f32)
            nc.gpsimd.partition_broadcast(crep[:], cgrow[:])

            for rt in range(ROWS_PER_CORE // 128):
                x1t = iop.tile([128, D], f32, tag="x1t")
                x2t = iop.tile([128, D], f32, tag="x2t")
                r0 = rt * 128
                nc.sync.dma_start(x1t[:], x1_d[r0:r0 + 128])
                nc.sync.dma_start(x2t[:], x2_d[r0:r0 + 128])

                sp = spp.tile([128, N_OPS * 64], f32, tag="sp")
                sp2 = spp.tile([128, N_OPS * 64], f32, tag="sp2")
                x13 = x1t[:].rearrange("p (b n) -> p b n", b=16)
                x23 = x2t[:].rearrange("p (b n) -> p b n", b=16)
                sp3 = sp[:].rearrange("p (o n) -> p o n", o=N_OPS)

                # pass A: block products, one TT per constant-delta run
                def bsl(ap3, b0, d, k):
                    if k == 1:
                        return ap3[:, b0:b0 + 1, :]
                    if d == 0:
                        return ap3[:, b0:b0 + 1, :].to_broadcast([128, k, 64])
                    if d > 0:
                        return ap3[:, b0:b0 + (k - 1) * d + 1:d, :]
                    stop = b0 + (k - 1) * d - 1
                    return ap3[:, b0:(stop if stop >= 0 else None):d, :]

                for (start, length, d1, d2) in RUNS:
                    b1, b2, _ = OPS[start]
                    nc.vector.tensor_mul(sp3[:, start:start + length, :],
                                         bsl(x13, b1, d1, length),
                                         bsl(x23, b2, d2, length))

                # pass B: scale by cg coefficients (replicated across partitions)
                nc.vector.tensor_mul(sp2[:], sp[:], crep[:])

                # pass C: segment reduce over ops, keeping the 64-wide ns dim
                outt = iop.tile([128, D], f32, tag="outt")
                sp23 = sp2[:].rearrange("p (o n) -> p o n", o=N_OPS)
                for bo in range(16):
                    s0, n_i = SEG[bo]
                    seg_ap = sp23[:, s0:s0 + n_i, :].transpose([0, 2, 1])
                    nc.vector.tensor_reduce(
                        outt[:, bo * 64:(bo + 1) * 64], seg_ap,
                        axis=mybir.AxisListType.X, op=mybir.AluOpType.add)
                nc.sync.dma_start(out_d[r0:r0 + 128], outt[:])

    nc.compile()
    return nc


def _get_nc():
    if "nc" not in _CACHE:
        _CACHE["nc"] = _build()
    return _CACHE["nc"]


def kernel(x1, x2, cg_tilde, repids_in1, repids_in2, repids_out, out_dim):
    from concourse.bass_utils import run_bass_kernel_spmd

    x1 = np.ascontiguousarray(np.asarray(x1, dtype=np.float32))
    x2 = np.ascontiguousarray(np.asarray(x2, dtype=np.float32))
    cg = np.asarray(cg_tilde, dtype=np.float32).reshape(N_OPS, 64)
    rid1 = np.asarray(repids_in1).reshape(N_OPS, 64)[:, 0] // 64
    rid2 = np.asarray(repids_in2).reshape(N_OPS, 64)[:, 0] // 64
    rido = np.asarray(repids_out).reshape(N_OPS, 64)[:, 0] // 64

    # map each hardcoded (b1,b2,bo) slot to its row in the runtime tables
    table = {}
    for k in range(N_OPS):
        table[(int(rid1[k]), int(rid2[k]), int(rido[k]))] = k
    order = np.array([table[op] for op in OPS], dtype=np.int64)
    cgrow = np.ascontiguousarray(cg[order].reshape(1, N_OPS * 64))

    nc = _get_nc()
    n = x1.shape[0]
    rows = n // N_CORES
    in_maps = []
    for k in range(N_CORES):
        sl = slice(k * rows, (k + 1) * rows)
        in_maps.append({
            "x1": np.ascontiguousarray(x1[sl]),
            "x2": np.ascontiguousarray(x2[sl]),
            "cgrow": cgrow,
        })
    res = run_bass_kernel_spmd(nc, in_maps, core_ids=list(range(N_CORES)))
    out = np.concatenate([res.results[k]["out"] for k in range(N_CORES)], axis=0)
    return out

